# revision 18
# baseline (speedup 1.0000x reference)
"""Causal self-attention (B=2, T=2048, C=1024, NH=16, D=64) on 8 TRN2 NeuronCores.

Sharding: 2-way batch x 4-way head-group tensor parallel (4 heads/core).
All matmuls bf16 with fp32 PSUM accumulation (fp8 fails the 2e-2 gate: any
fp8 in the value path costs ~2% output error since softmax averaging shrinks
signal and noise alike).  Speed comes from PE-array tiling:

- scores: the head pair runs CONCURRENTLY via row tiling -- head h occupies
  PE rows 64h..64h+63 (contraction D=64), so two [64,128]x[64,512] matmuls
  share one 512-cycle stream.
- att@v: the head pair runs concurrently via col tiling -- v_h0 at array
  cols 0-63 (psum partitions 0-63), v_h1 at cols 64-127.
- softmax denominators: four M=1 ones-column matmuls (one per (head, q-chunk))
  land at psum partitions {0,32,64,96} of one bank via explicit col
  tile_position; they run as one concurrent quad per k-tile and the
  reciprocal reads that PSUM bank directly (no 1-lane denominator copies).
- exp: one ACT per (k-tile, q-chunk) covers both heads' scores [128, 2, 512].

The host sums the 4 c_proj partials per batch (row-parallel reduction).
"""

import numpy as np
import ml_dtypes

import concourse.bass as bass
import concourse.mybir as mybir
import concourse.tile as tile
from concourse import bacc
from concourse.bass_utils import run_bass_kernel_spmd

BF16 = mybir.dt.bfloat16
F32 = mybir.dt.float32

B, T, C = 2, 2048, 1024
NH, D = 16, 64
HPC = NH // 4          # heads per core = 4
CL = HPC * D           # local channels = 256
N_CORES = 8

AF = mybir.ActivationFunctionType

NKT = C // 128        # 8 k-tiles over the C contraction
NTT = T // 128        # 16 t-tiles
NTC = T // 512        # 4 t-chunks
XPOS = {0: 0, 2: 1, 3: 2, 1: 3}   # physical chunk slot of logical t-chunk


def build_graph():
    nc = bacc.Bacc("TRN2")

    xT_d = nc.declare_dram_parameter("xT", [C, T], BF16, isOutput=False)
    wq_d = nc.declare_dram_parameter("wqT", [C, CL], BF16, isOutput=False)
    wk_d = nc.declare_dram_parameter("wkT", [C, CL], BF16, isOutput=False)
    wv_d = nc.declare_dram_parameter("wvT", [C, CL], BF16, isOutput=False)
    wp_d = nc.declare_dram_parameter("wpT", [CL, C], BF16, isOutput=False)
    bqk_d = nc.declare_dram_parameter("bqk", [128, 4], F32, isOutput=False)
    bv_d = nc.declare_dram_parameter("bv", [1, CL], BF16, isOutput=False)
    mask_d = nc.declare_dram_parameter("mask2", [128, 256], BF16, isOutput=False)
    out_d = nc.declare_dram_parameter("out", [C, T], BF16, isOutput=True)

    with tile.TileContext(nc) as tc:
        with (
            tc.tile_pool(name="persist", bufs=1) as pp,
            tc.tile_pool(name="work", bufs=6) as wpool,
            tc.tile_pool(name="bcast", bufs=2) as bcp,
            tc.tile_pool(name="dram", bufs=2, space="DRAM") as dpool,
        ):
            # ---- persistent SBUF tiles; DMA bandwidth is shared across
            # queues (~350GB/s aggregate), so the sync queue streams tensors in
            # exact first-need order: wk -> xT@tcn0 -> wq -> tcn2 -> tcn3 -> tcn1
            wq_sb = [pp.tile([128, CL], BF16, tag=f"wq{i}", name=f"wq{i}")
                     for i in range(NKT)]
            wk_sb = [pp.tile([128, CL], BF16, tag=f"wk{i}", name=f"wk{i}")
                     for i in range(NKT)]
            xT_sb = [pp.tile([128, T], BF16, tag=f"xT{i}", name=f"xT{i}")
                     for i in range(NKT)]
            wv_sb = [pp.tile([128, CL], BF16, tag=f"wv{i}", name=f"wv{i}")
                     for i in range(NKT)]
            for i in range(NKT):
                nc.scalar.dma_start(wk_sb[i][:], wk_d[128 * i : 128 * (i + 1), :])
            for i in range(NKT):
                nc.sync.dma_start(
                    xT_sb[i][:, 0:1536], xT_d[128 * i : 128 * (i + 1), 0:1536])
            for i in range(NKT):
                nc.scalar.dma_start(wv_sb[i][:], wv_d[128 * i : 128 * (i + 1), :])
            for i in range(NKT):
                nc.scalar.dma_start(wq_sb[i][:], wq_d[128 * i : 128 * (i + 1), :])
            for i in range(NKT):
                nc.gpsimd.dma_start(
                    xT_sb[i][:, 1536:2048],
                    xT_d[128 * i : 128 * (i + 1), 1536:2048])
            bqk_sb = pp.tile([128, 4], F32, tag="bqk")
            nc.scalar.dma_start(bqk_sb[:], bqk_d[:])
            bv_sb = pp.tile([1, CL], BF16, tag="bv")
            nc.scalar.dma_start(bv_sb[:], bv_d[:])
            wp_sb = [pp.tile([128, C], BF16, tag=f"wp{i}", name=f"wp{i}")
                     for i in range(CL // 128)]
            for i in range(CL // 128):
                nc.scalar.dma_start(wp_sb[i][:], wp_d[128 * i : 128 * (i + 1), :])
            mask_sb = pp.tile([128, 256], BF16, tag="mask")
            nc.scalar.dma_start(mask_sb[:], mask_d[:])
            mask_v = mask_sb[:].rearrange("p (h q) -> p h q", h=2)
            ones_sb = pp.tile([1, 128], BF16, tag="ones")
            nc.vector.memset(ones_sb[:], 1.0)
            onesc_sb = pp.tile([128, 1], BF16, tag="onesc")
            nc.vector.memset(onesc_sb[:], 1.0)
            onesb_sb = pp.tile([128, 64], F32, tag="onesb")
            nc.vector.memset(onesb_sb[:], 1.0)

            qkT_sb = [pp.tile([128, T], BF16, tag=f"qk{i}", name=f"qk{i}")
                      for i in range(4)]
            v_sb = [pp.tile([128, CL], BF16, tag=f"v{i}", name=f"v{i}")
                    for i in range(NTT)]
            yT_sb = [pp.tile([128, T], BF16, tag=f"y{i}", name=f"y{i}")
                     for i in range(CL // 128)]

            ps2_cm = tc.tile_pool(name="ps2", bufs=2, space="PSUM")
            ps2 = ps2_cm.__enter__()
            ps1_cm = tc.tile_pool(name="ps1", bufs=2, space="PSUM")
            ps1 = ps1_cm.__enter__()

            # ---- qk projection, feature-major: psum[f128, t512] ----
            def emit_qk(ft, tcn):
                pq = ps2.tile([128, 1024], F32, tag="S", name=f"pq{ft}{tcn}")
                w_sb = wq_sb if ft < 2 else wk_sb
                xp = 512 * XPOS[tcn]
                for kt in range(NKT):
                    nc.tensor.matmul(
                        pq[:, 0:512],
                        w_sb[kt][:, 128 * (ft % 2) : 128 * (ft % 2 + 1)],
                        xT_sb[kt][:, xp : xp + 512],
                        start=(kt == 0),
                        stop=(kt == NKT - 1),
                    )
                nc.vector.tensor_scalar_add(
                    qkT_sb[ft][:, 512 * tcn : 512 * (tcn + 1)],
                    pq[:, 0:512],
                    bqk_sb[:, ft : ft + 1],
                )

            # ---- v projection, t-major: psum[t128, 4h*64] ----
            def emit_v(tt):
                pv = ps2.tile([128, 1024], F32, tag="S", name=f"pv{tt}")
                tp = 512 * XPOS[tt // 4] + 128 * (tt % 4)
                for kt in range(NKT):
                    nc.tensor.matmul(
                        pv[:, 0:CL],
                        xT_sb[kt][:, tp : tp + 128],
                        wv_sb[kt][:],
                        start=(kt == 0),
                        stop=False,
                    )
                nc.tensor.matmul(
                    pv[:, 0:CL], ones_sb[:], bv_sb[:], start=False, stop=True
                )
                nc.vector.tensor_copy(v_sb[tt][:], pv[:, 0:CL])

            # ---- c_proj partial: out[o, t] += wpT^T @ yT ----
            def emit_cproj(tcn, mt):
                po = ps2.tile([128, 1024], F32, tag="S", name=f"po{mt}{tcn}")[:, 0:512]
                for ky in range(CL // 128):
                    nc.tensor.matmul(
                        po[:],
                        wp_sb[ky][:, 128 * mt : 128 * (mt + 1)],
                        yT_sb[ky][:, 512 * tcn : 512 * (tcn + 1)],
                        start=(ky == 0),
                        stop=(ky == CL // 128 - 1),
                    )
                ob = wpool.tile([128, 512], BF16, tag="ob", name=f"ob{mt}{tcn}")
                nc.vector.tensor_copy(ob[:], po[:])
                eng = (nc.gpsimd, nc.sync, nc.scalar)[(4 * mt + tcn) % 3] \
                    if tcn < 2 else (nc.gpsimd if (4 * mt + tcn) % 2 == 0 else nc.sync)
                eng.dma_start(
                    out_d[128 * mt : 128 * (mt + 1), 512 * tcn : 512 * (tcn + 1)],
                    ob[:],
                )

            # ---- attention for one (pass, head-pair) ----
            # S[128k, 2h, 512q] per (kt, qc); row-tiled concurrent scores;
            # one exp ACT for both heads; col-tiled concurrent av pair;
            # denominator quad at psum partitions {0,32,64,96} of one bank.
            fillers = []

            def attention(p, hp, fill_per_step):
                qf, kf = hp, 2 + hp
                qcs = (2, 3) if p == 1 else (0, 1)
                nkt = 16 if p == 1 else 8
                av = [ps1.tile([128, 512], F32, tag="av", name=f"av{p}{hp}{i}")
                      for i in range(2)]
                den = ps1.tile([128, 512], F32, tag="den", name=f"den{p}{hp}")
                nc.vector.memset(den[:], 1.0)
                def emit_avden(work):
                    for qci, qc, ns, E, ktp in work:
                        for h in (0, 1):
                            hh = 2 * hp + h
                            nc.tensor.matmul(
                                av[qci][64 * h : 64 * h + 64, ns:512],
                                v_sb[ktp][:, 64 * hh : 64 * hh + 64],
                                E[:, h, ns:512],
                                start=(ktp == 0),
                                stop=(ktp == 4 * qc + 3),
                                tile_position=(0, 64 * h),
                                skip_group_check=True,
                            )
                    for qci, qc, ns, E, ktp in work:
                        for h in (0, 1):
                            pos = 32 * (2 * qci + h)
                            nc.tensor.matmul(
                                den[pos : pos + 1, ns:512],
                                onesc_sb[:],
                                E[:, h, ns:512],
                                start=(ktp == 0),
                                stop=(ktp == 4 * qc + 3),
                                tile_position=(0, pos),
                                skip_group_check=True,
                            )

                pend = []
                for kt in range(nkt):
                    for _ in range(fill_per_step):
                        if fillers:
                            f, a = fillers.pop(0)
                            f(*a)
                    qc0 = kt // 4
                    so = 128 * kt - 512 * qc0
                    cur = []
                    for qci, qc in enumerate(qcs):
                        if qc < qc0:
                            continue
                        ns = so if qc == qc0 else 0
                        S = ps2.tile([128, 2, 512], F32, tag="S",
                                     name=f"S{p}{hp}{kt}{qci}")
                        for h in (0, 1):
                            nc.tensor.matmul(
                                S[:, h, ns:512],
                                qkT_sb[kf][64 * h : 64 * h + 64,
                                           128 * kt : 128 * (kt + 1)],
                                qkT_sb[qf][64 * h : 64 * h + 64,
                                           512 * qc + ns : 512 * (qc + 1)],
                                start=True,
                                stop=True,
                            )
                        E = wpool.tile([128, 1024], BF16, tag="E",
                                       name=f"E{p}{hp}{kt}{qci}")[:].rearrange(
                            "p (h q) -> p h q", h=2)
                        nc.scalar.activation(
                            E[:, :, ns:512], S[:, :, ns:512], AF.Exp, scale=0.125
                        )
                        if qc == qc0:
                            nc.vector.tensor_mul(
                                E[:, :, so : so + 128],
                                E[:, :, so : so + 128],
                                mask_v[:],
                            )
                        cur.append((qci, qc, ns, E, kt))
                    emit_avden(pend)
                    pend = cur
                emit_avden(pend)
                # normalization: reciprocal off the den bank, PE ones-matmul
                # partition-broadcast (no DRAM bounce), y = av * (1/den)
                rc = bcp.tile([97, 512], F32, tag="rc", name=f"rc{p}{hp}")
                nc.vector.reciprocal_approx_fast(out=rc[:], in_=den[0:97, :])
                for qci, qc in enumerate(qcs):
                    for h in (0, 1):
                        pos = 32 * (2 * qci + h)
                        bcp_ps = ps1.tile([64, 512], F32, tag="den",
                                          name=f"bc{p}{hp}{qci}{h}")
                        nc.tensor.matmul(
                            bcp_ps[:],
                            onesb_sb[pos : pos + 1, :],
                            rc[pos : pos + 1, :],
                            start=True,
                            stop=True,
                            tile_position=(pos, 0),
                            skip_group_check=True,
                        )
                        bcs = bcp.tile([64, 512], F32, tag="bcs",
                                       name=f"bcs{p}{hp}{qci}{h}")
                        nc.vector.tensor_copy(bcs[:], bcp_ps[:])
                        nc.vector.tensor_mul(
                            yT_sb[hp][64 * h : 64 * h + 64,
                                      512 * qc : 512 * (qc + 1)],
                            av[qci][64 * h : 64 * h + 64, :],
                            bcs[:],
                        )

            # lead-in: everything gating (1,0)'s start plus tcn0-gated chains
            # to keep the PE fed while the q-chunk tiles stream in
            emit_qk(2, 0)
            emit_v(0)
            emit_v(1)
            emit_qk(0, 2)
            emit_qk(0, 3)
            # (1,0) fillers, 2/step: v(tt) before step tt, kT chunk c before
            # step 4c, (1,1)'s gating chains by pass end
            fillers = [
                (emit_v, (2,)), (emit_v, (3,)),
                (emit_v, (4,)), (emit_v, (5,)),
                (emit_v, (6,)), (emit_v, (7,)),
                (emit_v, (8,)), (emit_qk, (2, 1)),
                (emit_v, (9,)), (emit_v, (10,)),
                (emit_v, (11,)), (emit_v, (12,)),
                (emit_v, (13,)), (emit_qk, (2, 2)),
                (emit_v, (14,)), (emit_v, (15,)),
                (emit_qk, (0, 0)), (emit_qk, (1, 0)),
                (emit_qk, (3, 0)), (emit_qk, (1, 2)),
                (emit_qk, (1, 3)), (emit_qk, (2, 3)),
            ]
            attention(1, 0, 2)
            # (1,1) fillers: its own late kT chunks + leftover projections
            fillers = [
                (emit_qk, (3, 1)), (emit_qk, (3, 2)),
                (emit_qk, (0, 1)), (emit_qk, (3, 3)),
                (emit_qk, (1, 1)),
            ]
            attention(1, 1, 1)
            fillers = [(emit_cproj, (2, mt)) for mt in range(8)]
            fillers += [(emit_cproj, (3, mt)) for mt in range(4)]
            attention(0, 0, 2)
            fillers = [(emit_cproj, (3, mt)) for mt in range(4, 8)]
            attention(0, 1, 1)
            while fillers:
                f, a = fillers.pop(0)
                f(*a)
            for i, (tcn, mt) in enumerate(
                [(tcn, mt) for tcn in (0, 1) for mt in range(8)]
            ):
                emit_cproj(tcn, mt)
            ps1_cm.__exit__(None, None, None)
            ps2_cm.__exit__(None, None, None)
    nc.finalize()
    return nc


_GRAPH_CACHE = {}


def kernel(x, W_attn, b_attn, W_proj, b_proj, bV, **_unused):
    x = np.asarray(x, dtype=np.float32)
    W_attn = np.asarray(W_attn, dtype=np.float32)
    b_attn = np.asarray(b_attn, dtype=np.float32)
    W_proj = np.asarray(W_proj, dtype=np.float32)
    b_proj = np.asarray(b_proj, dtype=np.float32)
    bV = np.asarray(bV, dtype=np.float32)

    bf = ml_dtypes.bfloat16
    perm = np.concatenate([np.arange(0, 512), np.arange(1024, 1536),
                           np.arange(1536, 2048), np.arange(512, 1024)])
    xT = [np.ascontiguousarray(x[b].T[:, perm]).astype(bf) for b in range(B)]
    m = np.triu(np.ones((128, 128), np.float32))
    mask2 = np.ascontiguousarray(np.concatenate([m, m], axis=1)).astype(bf)

    in_maps = []
    for core in range(N_CORES):
        b, g = core // 4, core % 4
        rq = slice(CL * g, CL * (g + 1))
        rk = slice(C + CL * g, C + CL * (g + 1))
        rv = slice(2 * C + CL * g, 2 * C + CL * (g + 1))
        wqT = np.ascontiguousarray(W_attn[rq].T).astype(bf)
        wkT = np.ascontiguousarray(W_attn[rk].T).astype(bf)
        wvT = np.ascontiguousarray(W_attn[rv].T).astype(bf)
        wpT = np.ascontiguousarray(W_proj[:, CL * g : CL * (g + 1)].T).astype(bf)
        bqk = np.concatenate([b_attn[rq], b_attn[rk]]).reshape(4, 128).T
        bqk = np.ascontiguousarray(bqk).astype(np.float32)
        bv = (bV[HPC * g : HPC * (g + 1)].reshape(1, CL) + b_attn[rv][None]).astype(bf)
        in_maps.append(
            {
                "xT": xT[b],
                "wqT": wqT,
                "wkT": wkT,
                "wvT": wvT,
                "wpT": wpT,
                "bqk": bqk,
                "bv": bv,
                "mask2": mask2,
            }
        )

    if "nc" not in _GRAPH_CACHE:
        _GRAPH_CACHE["nc"] = build_graph()
    nc = _GRAPH_CACHE["nc"]
    _GRAPH_CACHE["in_maps"] = in_maps

    res = run_bass_kernel_spmd(nc, in_maps, core_ids=list(range(N_CORES)))
    outs = [res.results[i]["out"] for i in range(N_CORES)]  # [C, T] partials

    out = np.empty((B, T, C), dtype=np.float32)
    for b in range(B):
        acc = outs[4 * b].astype(np.float32)
        for g in range(1, 4):
            acc += outs[4 * b + g].astype(np.float32)
        out[b] = acc.T + b_proj[None, :]
    return out


# revision 20
# speedup vs baseline: 1.0280x; 1.0280x over previous
"""Causal self-attention (B=2, T=2048, C=1024, NH=16, D=64) on 8 TRN2 NeuronCores.

Sharding: 2-way batch x 4-way head-group tensor parallel (4 heads/core).
All matmuls bf16 with fp32 PSUM accumulation (fp8 fails the 2e-2 gate: any
fp8 in the value path costs ~2% output error since softmax averaging shrinks
signal and noise alike).  Speed comes from PE-array tiling:

- scores: the head pair runs CONCURRENTLY via row tiling -- head h occupies
  PE rows 64h..64h+63 (contraction D=64), so two [64,128]x[64,512] matmuls
  share one 512-cycle stream.
- att@v: the head pair runs concurrently via col tiling -- v_h0 at array
  cols 0-63 (psum partitions 0-63), v_h1 at cols 64-127.
- softmax denominators: four M=1 ones-column matmuls (one per (head, q-chunk))
  land at psum partitions {0,32,64,96} of one bank via explicit col
  tile_position; they run as one concurrent quad per k-tile and the
  reciprocal reads that PSUM bank directly (no 1-lane denominator copies).
- exp: one ACT per (k-tile, q-chunk) covers both heads' scores [128, 2, 512].

The host sums the 4 c_proj partials per batch (row-parallel reduction).
"""

import numpy as np
import ml_dtypes

import concourse.bass as bass
import concourse.mybir as mybir
import concourse.tile as tile
from concourse import bacc
from concourse.bass_utils import run_bass_kernel_spmd

BF16 = mybir.dt.bfloat16
F32 = mybir.dt.float32

B, T, C = 2, 2048, 1024
NH, D = 16, 64
HPC = NH // 4          # heads per core = 4
CL = HPC * D           # local channels = 256
N_CORES = 8

AF = mybir.ActivationFunctionType

NKT = C // 128        # 8 k-tiles over the C contraction
NTT = T // 128        # 16 t-tiles
NTC = T // 512        # 4 t-chunks
XPOS = {0: 0, 2: 1, 3: 2, 1: 3}   # physical chunk slot of logical t-chunk


def build_graph():
    nc = bacc.Bacc("TRN2")

    xT_d = nc.declare_dram_parameter("xT", [C, T], BF16, isOutput=False)
    wq_d = nc.declare_dram_parameter("wqT", [C, CL], BF16, isOutput=False)
    wk_d = nc.declare_dram_parameter("wkT", [C, CL], BF16, isOutput=False)
    wv_d = nc.declare_dram_parameter("wvT", [C, CL], BF16, isOutput=False)
    wp_d = nc.declare_dram_parameter("wpT", [CL, C], BF16, isOutput=False)
    bqk_d = nc.declare_dram_parameter("bqk", [128, 4], F32, isOutput=False)
    bv_d = nc.declare_dram_parameter("bv", [1, CL], BF16, isOutput=False)
    mask_d = nc.declare_dram_parameter("mask2", [128, 256], BF16, isOutput=False)
    out_d = nc.declare_dram_parameter("out", [C, T], BF16, isOutput=True)

    with tile.TileContext(nc) as tc:
        with (
            tc.tile_pool(name="persist", bufs=1) as pp,
            tc.tile_pool(name="work", bufs=6) as wpool,
            tc.tile_pool(name="bcast", bufs=2) as bcp,
            tc.tile_pool(name="dram", bufs=2, space="DRAM") as dpool,
        ):
            # ---- persistent SBUF tiles; DMA bandwidth is shared across
            # queues (~350GB/s aggregate), so the sync queue streams tensors in
            # exact first-need order: wk -> xT@tcn0 -> wq -> tcn2 -> tcn3 -> tcn1
            wq_sb = [pp.tile([128, CL], BF16, tag=f"wq{i}", name=f"wq{i}")
                     for i in range(NKT)]
            wk_sb = [pp.tile([128, CL], BF16, tag=f"wk{i}", name=f"wk{i}")
                     for i in range(NKT)]
            xT_sb = [pp.tile([128, T], BF16, tag=f"xT{i}", name=f"xT{i}")
                     for i in range(NKT)]
            wv_sb = [pp.tile([128, CL], BF16, tag=f"wv{i}", name=f"wv{i}")
                     for i in range(NKT)]
            # per-queue DMA tops out near ~95GB/s; balance the 4.5MB the
            # attention start depends on across the three DMA-capable queues
            bqk_sb = pp.tile([128, 4], F32, tag="bqk")
            xa_eng = (nc.sync, nc.sync, nc.sync, nc.gpsimd,
                      nc.gpsimd, nc.gpsimd, nc.gpsimd, nc.scalar)
            for i in range(NKT):
                nc.sync.dma_start(wk_sb[i][:], wk_d[128 * i : 128 * (i + 1), :]) \
                    if i < 4 else nc.scalar.dma_start(
                        wk_sb[i][:], wk_d[128 * i : 128 * (i + 1), :])
            for i in range(NKT):
                nc.scalar.dma_start(wv_sb[i][:], wv_d[128 * i : 128 * (i + 1), :])
            nc.gpsimd.dma_start(bqk_sb[:], bqk_d[:])
            for i in range(NKT):
                xa_eng[i].dma_start(
                    xT_sb[i][:, 0:1536], xT_d[128 * i : 128 * (i + 1), 0:1536])
            for i in range(NKT):
                (nc.scalar if i < 4 else nc.gpsimd).dma_start(
                    wq_sb[i][:], wq_d[128 * i : 128 * (i + 1), :])
            bv_sb = pp.tile([1, CL], BF16, tag="bv")
            nc.scalar.dma_start(bv_sb[:], bv_d[:])
            for i in range(NKT):
                (nc.sync if i < 4 else nc.gpsimd).dma_start(
                    xT_sb[i][:, 1536:2048],
                    xT_d[128 * i : 128 * (i + 1), 1536:2048])
            wp_sb = [pp.tile([128, C], BF16, tag=f"wp{i}", name=f"wp{i}")
                     for i in range(CL // 128)]
            for i in range(CL // 128):
                nc.scalar.dma_start(wp_sb[i][:], wp_d[128 * i : 128 * (i + 1), :])
            mask_sb = pp.tile([128, 256], BF16, tag="mask")
            nc.scalar.dma_start(mask_sb[:], mask_d[:])
            mask_v = mask_sb[:].rearrange("p (h q) -> p h q", h=2)
            ones_sb = pp.tile([1, 128], BF16, tag="ones")
            nc.vector.memset(ones_sb[:], 1.0)
            onesc_sb = pp.tile([128, 1], BF16, tag="onesc")
            nc.vector.memset(onesc_sb[:], 1.0)
            onesb_sb = pp.tile([128, 64], F32, tag="onesb")
            nc.vector.memset(onesb_sb[:], 1.0)

            qkT_sb = [pp.tile([128, T], BF16, tag=f"qk{i}", name=f"qk{i}")
                      for i in range(4)]
            v_sb = [pp.tile([128, CL], BF16, tag=f"v{i}", name=f"v{i}")
                    for i in range(NTT)]
            yT_sb = [pp.tile([128, T], BF16, tag=f"y{i}", name=f"y{i}")
                     for i in range(CL // 128)]

            ps2_cm = tc.tile_pool(name="ps2", bufs=2, space="PSUM")
            ps2 = ps2_cm.__enter__()
            ps1_cm = tc.tile_pool(name="ps1", bufs=2, space="PSUM")
            ps1 = ps1_cm.__enter__()

            # ---- qk projection, feature-major: psum[f128, t512] ----
            def emit_qk(ft, tcn):
                pq = ps2.tile([128, 1024], F32, tag="S", name=f"pq{ft}{tcn}")
                w_sb = wq_sb if ft < 2 else wk_sb
                xp = 512 * XPOS[tcn]
                for kt in range(NKT):
                    nc.tensor.matmul(
                        pq[:, 0:512],
                        w_sb[kt][:, 128 * (ft % 2) : 128 * (ft % 2 + 1)],
                        xT_sb[kt][:, xp : xp + 512],
                        start=(kt == 0),
                        stop=(kt == NKT - 1),
                    )
                nc.vector.tensor_scalar_add(
                    qkT_sb[ft][:, 512 * tcn : 512 * (tcn + 1)],
                    pq[:, 0:512],
                    bqk_sb[:, ft : ft + 1],
                )

            # ---- v projection, t-major: psum[t128, 4h*64] ----
            def emit_v(tt):
                pv = ps2.tile([128, 1024], F32, tag="S", name=f"pv{tt}")
                tp = 512 * XPOS[tt // 4] + 128 * (tt % 4)
                for kt in range(NKT):
                    nc.tensor.matmul(
                        pv[:, 0:CL],
                        xT_sb[kt][:, tp : tp + 128],
                        wv_sb[kt][:],
                        start=(kt == 0),
                        stop=False,
                    )
                nc.tensor.matmul(
                    pv[:, 0:CL], ones_sb[:], bv_sb[:], start=False, stop=True
                )
                nc.vector.tensor_copy(v_sb[tt][:], pv[:, 0:CL])

            # ---- c_proj partial: out[o, t] += wpT^T @ yT ----
            def emit_cproj(tcn, mt):
                po = ps2.tile([128, 1024], F32, tag="S", name=f"po{mt}{tcn}")[:, 0:512]
                for ky in range(CL // 128):
                    nc.tensor.matmul(
                        po[:],
                        wp_sb[ky][:, 128 * mt : 128 * (mt + 1)],
                        yT_sb[ky][:, 512 * tcn : 512 * (tcn + 1)],
                        start=(ky == 0),
                        stop=(ky == CL // 128 - 1),
                    )
                ob = wpool.tile([128, 512], BF16, tag="ob", name=f"ob{mt}{tcn}")
                nc.vector.tensor_copy(ob[:], po[:])
                eng = (nc.gpsimd, nc.sync, nc.scalar)[(4 * mt + tcn) % 3] \
                    if tcn < 2 else (nc.gpsimd if (4 * mt + tcn) % 2 == 0 else nc.sync)
                eng.dma_start(
                    out_d[128 * mt : 128 * (mt + 1), 512 * tcn : 512 * (tcn + 1)],
                    ob[:],
                )

            # ---- attention for one (pass, head-pair) ----
            # S[128k, 2h, 512q] per (kt, qc); row-tiled concurrent scores;
            # one exp ACT for both heads; col-tiled concurrent av pair;
            # denominator quad at psum partitions {0,32,64,96} of one bank.
            fillers = []

            def attention(p, hp, fill_per_step):
                qf, kf = hp, 2 + hp
                qcs = (2, 3) if p == 1 else (0, 1)
                nkt = 16 if p == 1 else 8
                av = [ps1.tile([128, 512], F32, tag="av", name=f"av{p}{hp}{i}")
                      for i in range(2)]
                den = ps1.tile([128, 512], F32, tag="den", name=f"den{p}{hp}")
                nc.vector.memset(den[:], 1.0)
                def emit_avden(work):
                    for qci, qc, ns, E, ktp in work:
                        for h in (0, 1):
                            hh = 2 * hp + h
                            nc.tensor.matmul(
                                av[qci][64 * h : 64 * h + 64, ns:512],
                                v_sb[ktp][:, 64 * hh : 64 * hh + 64],
                                E[:, h, ns:512],
                                start=(ktp == 0),
                                stop=(ktp == 4 * qc + 3),
                                tile_position=(0, 64 * h),
                                skip_group_check=True,
                            )
                    for qci, qc, ns, E, ktp in work:
                        for h in (0, 1):
                            pos = 32 * (2 * qci + h)
                            nc.tensor.matmul(
                                den[pos : pos + 1, ns:512],
                                onesc_sb[:],
                                E[:, h, ns:512],
                                start=(ktp == 0),
                                stop=(ktp == 4 * qc + 3),
                                tile_position=(0, pos),
                                skip_group_check=True,
                            )

                pend = []
                for kt in range(nkt):
                    for _ in range(fill_per_step):
                        if fillers:
                            f, a = fillers.pop(0)
                            f(*a)
                    qc0 = kt // 4
                    so = 128 * kt - 512 * qc0
                    cur = []
                    for qci, qc in enumerate(qcs):
                        if qc < qc0:
                            continue
                        ns = so if qc == qc0 else 0
                        S = ps2.tile([128, 2, 512], F32, tag="S",
                                     name=f"S{p}{hp}{kt}{qci}")
                        for h in (0, 1):
                            nc.tensor.matmul(
                                S[:, h, ns:512],
                                qkT_sb[kf][64 * h : 64 * h + 64,
                                           128 * kt : 128 * (kt + 1)],
                                qkT_sb[qf][64 * h : 64 * h + 64,
                                           512 * qc + ns : 512 * (qc + 1)],
                                start=True,
                                stop=True,
                            )
                        E = wpool.tile([128, 1024], BF16, tag="E",
                                       name=f"E{p}{hp}{kt}{qci}")[:].rearrange(
                            "p (h q) -> p h q", h=2)
                        nc.scalar.activation(
                            E[:, :, ns:512], S[:, :, ns:512], AF.Exp, scale=0.125
                        )
                        if qc == qc0:
                            nc.vector.tensor_mul(
                                E[:, :, so : so + 128],
                                E[:, :, so : so + 128],
                                mask_v[:],
                            )
                        cur.append((qci, qc, ns, E, kt))
                    emit_avden(pend)
                    pend = cur
                emit_avden(pend)
                # normalization: reciprocal off the den bank, PE ones-matmul
                # partition-broadcast (no DRAM bounce), y = av * (1/den)
                rc = bcp.tile([97, 512], F32, tag="rc", name=f"rc{p}{hp}")
                nc.vector.reciprocal_approx_fast(out=rc[:], in_=den[0:97, :])
                for qci, qc in enumerate(qcs):
                    for h in (0, 1):
                        pos = 32 * (2 * qci + h)
                        bcp_ps = ps1.tile([64, 512], F32, tag="den",
                                          name=f"bc{p}{hp}{qci}{h}")
                        nc.tensor.matmul(
                            bcp_ps[:],
                            onesb_sb[pos : pos + 1, :],
                            rc[pos : pos + 1, :],
                            start=True,
                            stop=True,
                            tile_position=(pos, 0),
                            skip_group_check=True,
                        )
                        bcs = bcp.tile([64, 512], F32, tag="bcs",
                                       name=f"bcs{p}{hp}{qci}{h}")
                        nc.vector.tensor_copy(bcs[:], bcp_ps[:])
                        nc.vector.tensor_mul(
                            yT_sb[hp][64 * h : 64 * h + 64,
                                      512 * qc : 512 * (qc + 1)],
                            av[qci][64 * h : 64 * h + 64, :],
                            bcs[:],
                        )

            # lead-in: everything gating (1,0)'s start plus tcn0-gated chains
            # to keep the PE fed while the q-chunk tiles stream in
            emit_qk(2, 0)
            emit_v(0)
            emit_v(1)
            emit_qk(0, 2)
            emit_qk(0, 3)
            # (1,0) fillers, 2/step: v(tt) before step tt, kT chunk c before
            # step 4c, (1,1)'s gating chains by pass end
            fillers = [
                (emit_v, (2,)), (emit_v, (3,)),
                (emit_v, (8,)), (emit_v, (9,)),
                (emit_v, (10,)), (emit_v, (11,)),
                (emit_v, (12,)), (emit_v, (13,)),
                (emit_v, (14,)), (emit_v, (15,)),
                (emit_qk, (2, 1)), (emit_v, (4,)),
                (emit_v, (5,)), (emit_v, (6,)),
                (emit_v, (7,)), (emit_qk, (2, 2)),
                (emit_qk, (3, 0)), (emit_qk, (1, 2)),
                (emit_qk, (1, 3)), (emit_qk, (2, 3)),
                (emit_qk, (0, 0)), (emit_qk, (1, 0)),
            ]
            attention(1, 0, 2)
            # (1,1) fillers: its own late kT chunks + leftover projections
            fillers = [
                (emit_qk, (3, 1)), (emit_qk, (3, 2)),
                (emit_qk, (0, 1)), (emit_qk, (3, 3)),
                (emit_qk, (1, 1)),
            ]
            attention(1, 1, 1)
            fillers = [(emit_cproj, (2, mt)) for mt in range(8)]
            fillers += [(emit_cproj, (3, mt)) for mt in range(4)]
            attention(0, 0, 2)
            fillers = [(emit_cproj, (3, mt)) for mt in range(4, 8)]
            attention(0, 1, 1)
            while fillers:
                f, a = fillers.pop(0)
                f(*a)
            for i, (tcn, mt) in enumerate(
                [(tcn, mt) for tcn in (0, 1) for mt in range(8)]
            ):
                emit_cproj(tcn, mt)
            ps1_cm.__exit__(None, None, None)
            ps2_cm.__exit__(None, None, None)
    nc.finalize()
    return nc


_GRAPH_CACHE = {}


def kernel(x, W_attn, b_attn, W_proj, b_proj, bV, **_unused):
    x = np.asarray(x, dtype=np.float32)
    W_attn = np.asarray(W_attn, dtype=np.float32)
    b_attn = np.asarray(b_attn, dtype=np.float32)
    W_proj = np.asarray(W_proj, dtype=np.float32)
    b_proj = np.asarray(b_proj, dtype=np.float32)
    bV = np.asarray(bV, dtype=np.float32)

    bf = ml_dtypes.bfloat16
    perm = np.concatenate([np.arange(0, 512), np.arange(1024, 1536),
                           np.arange(1536, 2048), np.arange(512, 1024)])
    xT = [np.ascontiguousarray(x[b].T[:, perm]).astype(bf) for b in range(B)]
    m = np.triu(np.ones((128, 128), np.float32))
    mask2 = np.ascontiguousarray(np.concatenate([m, m], axis=1)).astype(bf)

    in_maps = []
    for core in range(N_CORES):
        b, g = core // 4, core % 4
        rq = slice(CL * g, CL * (g + 1))
        rk = slice(C + CL * g, C + CL * (g + 1))
        rv = slice(2 * C + CL * g, 2 * C + CL * (g + 1))
        wqT = np.ascontiguousarray(W_attn[rq].T).astype(bf)
        wkT = np.ascontiguousarray(W_attn[rk].T).astype(bf)
        wvT = np.ascontiguousarray(W_attn[rv].T).astype(bf)
        wpT = np.ascontiguousarray(W_proj[:, CL * g : CL * (g + 1)].T).astype(bf)
        bqk = np.concatenate([b_attn[rq], b_attn[rk]]).reshape(4, 128).T
        bqk = np.ascontiguousarray(bqk).astype(np.float32)
        bv = (bV[HPC * g : HPC * (g + 1)].reshape(1, CL) + b_attn[rv][None]).astype(bf)
        in_maps.append(
            {
                "xT": xT[b],
                "wqT": wqT,
                "wkT": wkT,
                "wvT": wvT,
                "wpT": wpT,
                "bqk": bqk,
                "bv": bv,
                "mask2": mask2,
            }
        )

    if "nc" not in _GRAPH_CACHE:
        _GRAPH_CACHE["nc"] = build_graph()
    nc = _GRAPH_CACHE["nc"]
    _GRAPH_CACHE["in_maps"] = in_maps

    res = run_bass_kernel_spmd(nc, in_maps, core_ids=list(range(N_CORES)))
    outs = [res.results[i]["out"] for i in range(N_CORES)]  # [C, T] partials

    out = np.empty((B, T, C), dtype=np.float32)
    for b in range(B):
        acc = outs[4 * b].astype(np.float32)
        for g in range(1, 4):
            acc += outs[4 * b + g].astype(np.float32)
        out[b] = acc.T + b_proj[None, :]
    return out
